# revision 29
# baseline (speedup 1.0000x reference)
"""Trainium2 Bass kernel for nn_Blast: out = x @ (W0 + 1 bias^T) + bias
where W0 block (i_in, i_out) = Vt[i] @ diag(S[o,i]) @ U[o].

Two-stage factorized algorithm (per core, 256 tokens, all fp16 matmuls):
  step1: y[(i,r), tok] = Vt_aug[i]^T @ x_i          (64 thin matmuls, M=32,
         K=128, 4-way concurrent via PE col-tiling at tile_position (0,32j))
  step2: mid[(o,r), tok] = Shat^T @ y               (16 full matmuls K=128)
  phaseB: out[tok, oq] = mid_o^T @ U''[o]           (K=18 row-tiled like the
         f32r baseline, 4-way concurrent)

Shat is the block-sparse S matrix: Shat_m[32j+r, 128g+32j'+r'] =
S[o=4g+j', i=4j+m, r] * delta(r,r'), built on device as
broadcast(S') * D with D the 32-diagonal 0/1 mask (shipped, 128KB) and
S' the 128x64 repacked S (16KB).

Bias trick: out = x@W0 + (rowsum(x)+1)*bias.  Vt_aug has a 17th ones
column -> y row (i,16) = block rowsum; Shat maps sum_i -> mid row (o,16) =
full rowsum; each mid bank is opened by a K=1 ones-matmul so every mid row
carries +1; U'' row 16 = bias (multiplies rowsum+1), row 17 = -sum_r U
(multiplies the constant 1.0 padding row, cancelling the rank-row
pollution).

Everything is fp16: x in (2MB/core), out back (2MB/core); factors ~0.6MB.
DMA-bound: x + warm seed on the sync HWDGE queue, factors on the gpsimd
SWDGE queue, output chunks on the scalar HWDGE queue.  Token-halves
pipeline (step1/2/B per 128-token half) overlaps the output DMA of half 0
with the compute of half 1.

PE warmup: HAM unthrottles 1.2->2.4 GHz after ~3.4us of sustained full-K
matmul activity; ~24 dummy matmuls run during the x-DMA window.

Sharding: pure data-parallel over the 2048 tokens (8 cores x 256); the
small factors are replicated.
"""

import numpy as np

IN_DIM = 4096
OUT_DIM = 4096
BLOCK = 256
RANK = 16
B_IN = 16
B_OUT = 16
N_CORES = 8
TOK = 2048
TPC = TOK // N_CORES          # 256 tokens per core
HT = 128                      # tokens per half
CP = 32                       # per-block slot width (PE 32-row groups)
KU = RANK + 2                 # 18 used rows of U'' per o-block
NWARM = 44

_CACHE = {}

# test.py toggles; harness never touches these
TRACE = False
TRACE_DIR = None
LAST_RESULTS = None


def build_program():
    import concourse.mybir as mybir
    from concourse import bacc
    from concourse.tile import TileContext

    f16 = mybir.dt.float16
    f32 = mybir.dt.float32

    nc = bacc.Bacc(trn_type="TRN2")
    xt_d = nc.dram_tensor("xt", (2, 4, 128, 8 * HT), f16, kind="ExternalInput")
    # vt ships first on the sync queue (step 1 needs it before anything);
    # the rest coalesce into one gpsimd DMA:
    # cols [0:64] sprime | [64:576] dmask | [576:1600] usb2 | [1600:1856] ones
    vt_d = nc.dram_tensor("vt", (128, 1024), f16, kind="ExternalInput")
    fac_d = nc.dram_tensor("fac", (128, 1856), f16, kind="ExternalInput")
    out_d = nc.dram_tensor("out", (2, HT, OUT_DIM), f16, kind="ExternalOutput")

    with TileContext(nc) as tc:
        from contextlib import ExitStack

        with ExitStack() as ctx:
            consts = ctx.enter_context(tc.tile_pool(name="consts", bufs=1))
            xpool = ctx.enter_context(tc.tile_pool(name="xpool", bufs=1))
            spool = ctx.enter_context(tc.tile_pool(name="spool", bufs=1))
            ypool = ctx.enter_context(tc.tile_pool(name="ypool", bufs=2))
            mpool = ctx.enter_context(tc.tile_pool(name="mpool", bufs=2))
            osbp = ctx.enter_context(tc.tile_pool(name="osbp", bufs=6))
            # PSUM is 8 banks of [128, 512] f32; every tile is bank-padded.
            # phase-B pairs live here (4 banks); y/mid pools (4 banks each)
            # are scoped per phase below and time-share the other 4 banks.
            ps_o = ctx.enter_context(tc.tile_pool(name="ps_o", bufs=1, space="PSUM"))

            # ---- input DMAs ----
            # vt first, then x as 8 x 256KB quarter-chunks on the sync
            # queue; step-1 quad q consumes exactly chunk (h, q)
            vt_sb = consts.tile([128, 1024], f16, name="vt", tag="vt")
            nc.sync.dma_start(out=vt_sb[:], in_=vt_d[:])
            vt_v = vt_sb[:].rearrange("p (i c r) -> p i c r", i=B_IN, c=2)
            xb = []
            for h in range(2):
                xt = xpool.tile([128, 32 * HT], f16, name=f"xb{h}", tag=f"xb{h}")
                for qd in range(4):
                    eng = nc.sync if qd % 2 == 0 else nc.scalar
                    eng.dma_start(
                        out=xt[:, qd * 8 * HT : (qd + 1) * 8 * HT],
                        in_=xt_d[h, qd],
                    )
                xb.append(xt)

            # gpsimd SWDGE queue: one coalesced factor DMA
            fac_sb = consts.tile([128, 1856], f16, name="fac", tag="fac")
            nc.gpsimd.dma_start(out=fac_sb[:], in_=fac_d[:])
            sp_sb = fac_sb[:, 0:64]
            dsb = fac_sb[:, 64:576]
            usb2 = fac_sb[:, 576:1600]    # usb2[32j+r, g*256+q] = U''[4g+j,r,q]
            kones = fac_sb[0:1, 1600:1856]

            # ---- Shat build on DVE: Shat_m = broadcast(S'_m) * D ----
            ssb = []
            for m in range(4):
                st = spool.tile([128, 4 * 128], f16, name=f"ss{m}", tag=f"ss{m}")
                nc.vector.tensor_mul(
                    st[:].rearrange("p (c k) -> p c k", k=CP),
                    sp_sb[:, m * B_OUT : (m + 1) * B_OUT]
                    .unsqueeze(2)
                    .broadcast_to([128, B_OUT, CP]),
                    dsb[:].rearrange("p (c k) -> p c k", k=CP),
                )
                ssb.append(st)

            # ---- step 1 per half; step 2 unified over both halves ----
            ysb = [
                ypool.tile([128, 2 * HT], f16, name=f"ys{m}", tag=f"ys{m}")
                for m in range(4)
            ]
            for h in range(2):
                with tc.tile_pool(name=f"psy{h}", bufs=1, space="PSUM") as ps_yh:
                    # step 1: y[(i,r), t] in 4 psum tiles (1 bank each), 4
                    # col slots; quad q consumes x chunks 8q..8q+7, and its
                    # 4 back-to-back matmuls hit 4 distinct banks AND 4
                    # distinct PE col-groups (concurrent subarray tiles).
                    yp = [
                        ps_yh.tile([128, HT], f32, name=f"y{h}{m}", tag=f"y{m}")
                        for m in range(4)
                    ]
                    for q in range(4):
                        for c in range(2):
                            for m in range(4):
                                j = (m + q) % 4
                                i = 4 * q + m
                                k = 2 * i + c
                                nc.tensor.matmul(
                                    yp[m][32 * j : 32 * j + 32, :],
                                    lhsT=vt_v[:, i, c, :],
                                    rhs=xb[h][:, k * HT : (k + 1) * HT],
                                    start=(c == 0),
                                    stop=(c == 1),
                                    tile_position=(0, 32 * j),
                                )

                    # y -> SBUF fp16 (token-half columns), DVE + ACT split
                    for m in range(4):
                        sl = (slice(None), slice(h * HT, (h + 1) * HT))
                        if m % 2 == 0:
                            nc.vector.tensor_copy(ysb[m][sl], yp[m][:])
                        else:
                            nc.scalar.copy(ysb[m][sl], yp[m][:])

            with tc.tile_pool(name="psm", bufs=1, space="PSUM") as ps_m:
                # step 2 over all 256 tokens: mid[g] += Shat_m^T @ y_m.
                # Each mid bank is opened by a ones-matmul writing 1.0
                # everywhere: rank rows carry mid+1 (cancelled by U'' row 17
                # = -sum_r U via the 1.0 padding row), the rowsum row
                # carries rowsum+1 (exactly what the bias needs).
                mp = [
                    ps_m.tile([128, 2 * HT], f32, name=f"mp{g}", tag=f"mp{g}")
                    for g in range(4)
                ]
                for g in range(4):
                    nc.tensor.matmul(
                        mp[g][:],
                        lhsT=kones[:, 0:128],
                        rhs=kones[:],
                        start=True,
                        stop=False,
                        tile_position=(0, 0),
                    )
                for m in range(4):
                    for g in range(4):
                        nc.tensor.matmul(
                            mp[g][:],
                            lhsT=ssb[m][:, g * 128 : (g + 1) * 128],
                            rhs=ysb[m][:],
                            start=False,
                            stop=(m == 3),
                            tile_position=(0, 0),
                        )

                mids = [
                    mpool.tile([128, 2 * HT], f16, name=f"ms{g}", tag=f"ms{g}")
                    for g in range(4)
                ]
                for g in range(4):
                    if g % 2 == 0:
                        nc.vector.tensor_copy(mids[g][:], mp[g][:])
                    else:
                        nc.scalar.copy(mids[g][:], mp[g][:])

            # phase B per half: out tiles [128 tok, 256 q], K=18 row-tiled;
            # the 4 j-slots of one mids tile run concurrently, one psum
            # bank each; per-g 256KB chunks stream out on both HWDGE queues.
            for h in range(2):
                osb_t = osbp.tile(
                    [128, B_OUT * BLOCK], f16, name=f"osb{h}", tag="osb", bufs=2
                )
                for g in range(4):
                    pos = [
                        ps_o.tile([128, BLOCK], f32, name=f"po{j}", tag=f"po{j}")
                        for j in range(4)
                    ]
                    for j in range(4):
                        nc.tensor.matmul(
                            pos[j][:],
                            lhsT=mids[g][
                                32 * j : 32 * j + KU, h * HT : (h + 1) * HT
                            ],
                            rhs=usb2[32 * j : 32 * j + KU, g * BLOCK : (g + 1) * BLOCK],
                            start=True,
                            stop=True,
                            tile_position=(32 * j, 0),
                        )
                    for j in range(4):
                        if j % 2 == 0:
                            nc.vector.tensor_copy(
                                osb_t[:, (4 * g + j) * BLOCK : (4 * g + j + 1) * BLOCK],
                                pos[j][:],
                            )
                        else:
                            nc.scalar.copy(
                                osb_t[:, (4 * g + j) * BLOCK : (4 * g + j + 1) * BLOCK],
                                pos[j][:],
                            )
                    eng = nc.sync if g % 2 == 0 else nc.scalar
                    eng.dma_start(
                        out=out_d[h][:, g * 4 * BLOCK : (g + 1) * 4 * BLOCK],
                        in_=osb_t[:, g * 4 * BLOCK : (g + 1) * 4 * BLOCK],
                    )

    nc.compile()
    return nc


def prep_inputs(x, S, U, Vt, bias):
    """Host-side layout prep. Returns per-core input maps."""
    x = np.asarray(x, dtype=np.float32)
    S = np.asarray(S, dtype=np.float32)
    U = np.asarray(U, dtype=np.float32)
    Vt = np.asarray(Vt, dtype=np.float32)
    bias = np.asarray(bias, dtype=np.float32)

    # x -> per-core [half, quarter, p, k*HT + t], in = 128k+p, tok = 256c+128h+t
    x2 = x.reshape(TOK, IN_DIM).astype(np.float16)
    # [c, h, t, k, p] -> [c, h, p, k, t] -> quarter chunks [c, h, qd, p, kq, t]
    xt5 = x2.reshape(N_CORES, 2, HT, 32, 128).transpose(0, 1, 4, 3, 2)
    xt5 = np.ascontiguousarray(xt5).reshape(N_CORES, 2, 128, 4, 8 * HT)
    xt5 = np.ascontiguousarray(xt5.transpose(0, 1, 3, 2, 4))

    # Vt_aug: [p, (i, c, r32)]; col 16 = ones (rowsum), cols 17..31 = 0
    vt_aug = np.zeros((B_IN, BLOCK, CP), np.float32)
    vt_aug[:, :, :RANK] = Vt
    vt_aug[:, :, RANK] = 1.0
    vt_host = vt_aug.reshape(B_IN, 2, 128, CP).transpose(2, 0, 1, 3).reshape(128, -1)

    # S': sp[32j+r, 16m+o] = S[o, i(m,j), r] (r<16) with i = 4q+m,
    # j = (m+q)%4 (quad q of step 1 consumes x chunks 8q..8q+7);
    # row r=16 all ones; rest 0.
    sp = np.zeros((4, CP, 4, B_OUT), np.float32)  # [j, r32, m, o]
    for m in range(4):
        for q in range(4):
            j = (m + q) % 4
            i = 4 * q + m
            sp[j, :RANK, m, :] = S[:, i, :].T
    sp[:, RANK] = 1.0
    sp_host = sp.reshape(128, 4 * B_OUT)

    # D mask: D[p, c] = 1 if p%32 == c%32 and p%32 <= 17
    pp = np.arange(128) % CP
    cc = np.arange(512) % CP
    dmask = ((pp[:, None] == cc[None, :]) & (pp[:, None] <= RANK + 1)).astype(
        np.float32
    )

    # U'' rows: 16 rank rows, bias (x rowsum+1), -sum_r U (cancels the +1
    # mid-bank-open pollution via the constant-1.0 padding row)
    bias_row = bias.reshape(B_OUT, 1, BLOCK)
    comp_row = -U.sum(axis=1, keepdims=True)
    u_aug = np.concatenate([U, bias_row, comp_row], axis=1)  # (16, 18, 256)

    # usb2[32j+r, g*256+q] = U''[o=4g+j, r, q]
    usb2 = np.zeros((4, CP, 4, BLOCK), np.float32)
    for j in range(4):
        for g in range(4):
            usb2[j, :KU, g] = u_aug[4 * g + j]
    usb2 = usb2.reshape(128, 4 * BLOCK)

    fac = np.zeros((128, 1856), np.float32)
    fac[:, 0:64] = sp_host
    fac[:, 64:576] = dmask
    fac[:, 576:1600] = usb2
    fac[0, 1600:1856] = 1.0
    fac = np.ascontiguousarray(fac).astype(np.float16)
    vt_ship = np.ascontiguousarray(vt_host).astype(np.float16)


    in_maps = []
    for c in range(N_CORES):
        in_maps.append(
            {
                "xt": np.ascontiguousarray(xt5[c]),
                "fac": fac,
                "vt": vt_ship,
            }
        )
    return in_maps


def kernel(x, S, U, Vt, bias):
    global LAST_RESULTS
    from concourse.bass_utils import run_bass_kernel_spmd

    if "nc" not in _CACHE:
        _CACHE["nc"] = build_program()
    nc = _CACHE["nc"]

    in_maps = prep_inputs(x, S, U, Vt, bias)
    res = run_bass_kernel_spmd(
        nc, in_maps, list(range(N_CORES)), trace=TRACE, tmpdir=TRACE_DIR
    )
    LAST_RESULTS = res
    out = np.concatenate(
        [res.results[c]["out"].reshape(TPC, OUT_DIM) for c in range(N_CORES)], axis=0
    ).astype(np.float32)
    return out.reshape(2, TOK // 2, OUT_DIM)


# revision 30
# speedup vs baseline: 1.0704x; 1.0704x over previous
"""Trainium2 Bass kernel for nn_Blast: out = x @ (W0 + 1 bias^T) + bias
where W0 block (i_in, i_out) = Vt[i] @ diag(S[o,i]) @ U[o].

Two-stage factorized algorithm (per core, 256 tokens, all fp16 matmuls):
  step1: y[(i,r), tok] = Vt_aug[i]^T @ x_i          (64 thin matmuls, M=32,
         K=128, 4-way concurrent via PE col-tiling at tile_position (0,32j))
  step2: mid[(o,r), tok] = Shat^T @ y               (16 full matmuls K=128)
  phaseB: out[tok, oq] = mid_o^T @ U''[o]           (K=18 row-tiled like the
         f32r baseline, 4-way concurrent)

Shat is the block-sparse S matrix: Shat_m[32j+r, 128g+32j'+r'] =
S[o=4g+j', i=4j+m, r] * delta(r,r'), built on device as
broadcast(S') * D with D the 32-diagonal 0/1 mask (shipped, 128KB) and
S' the 128x64 repacked S (16KB).

Bias trick: out = x@W0 + (rowsum(x)+1)*bias.  Vt_aug has a 17th ones
column -> y row (i,16) = block rowsum; Shat maps sum_i -> mid row (o,16) =
full rowsum; each mid bank is opened by a K=1 ones-matmul so every mid row
carries +1; U'' row 16 = bias (multiplies rowsum+1), row 17 = -sum_r U
(multiplies the constant 1.0 padding row, cancelling the rank-row
pollution).

Everything is fp16: x in (2MB/core), out back (2MB/core); factors ~0.6MB.
DMA-bound: x + warm seed on the sync HWDGE queue, factors on the gpsimd
SWDGE queue, output chunks on the scalar HWDGE queue.  Token-halves
pipeline (step1/2/B per 128-token half) overlaps the output DMA of half 0
with the compute of half 1.

PE warmup: HAM unthrottles 1.2->2.4 GHz after ~3.4us of sustained full-K
matmul activity; ~24 dummy matmuls run during the x-DMA window.

Sharding: pure data-parallel over the 2048 tokens (8 cores x 256); the
small factors are replicated.
"""

import numpy as np

IN_DIM = 4096
OUT_DIM = 4096
BLOCK = 256
RANK = 16
B_IN = 16
B_OUT = 16
N_CORES = 8
TOK = 2048
TPC = TOK // N_CORES          # 256 tokens per core
HT = 128                      # tokens per half
CP = 32                       # per-block slot width (PE 32-row groups)
KU = RANK + 2                 # 18 used rows of U'' per o-block
NWARM = 44

_CACHE = {}

# test.py toggles; harness never touches these
TRACE = False
TRACE_DIR = None
LAST_RESULTS = None


def build_program():
    import concourse.mybir as mybir
    from concourse import bacc
    from concourse.tile import TileContext

    f16 = mybir.dt.float16
    f32 = mybir.dt.float32

    nc = bacc.Bacc(trn_type="TRN2")
    xt_d = nc.dram_tensor("xt", (2, 4, 128, 8 * HT), f16, kind="ExternalInput")
    # vt ships first on the sync queue (step 1 needs it before anything);
    # the rest coalesce into one gpsimd DMA:
    # cols [0:64] sprime | [64:576] dmask | [576:1600] usb2 | [1600:1856] ones
    vt_d = nc.dram_tensor("vt", (128, 1024), f16, kind="ExternalInput")
    fac_d = nc.dram_tensor("fac", (128, 1856), f16, kind="ExternalInput")
    out_d = nc.dram_tensor("out", (2, HT, OUT_DIM), f16, kind="ExternalOutput")

    with TileContext(nc) as tc:
        from contextlib import ExitStack

        with ExitStack() as ctx:
            consts = ctx.enter_context(tc.tile_pool(name="consts", bufs=1))
            xpool = ctx.enter_context(tc.tile_pool(name="xpool", bufs=1))
            spool = ctx.enter_context(tc.tile_pool(name="spool", bufs=1))
            ypool = ctx.enter_context(tc.tile_pool(name="ypool", bufs=2))
            mpool = ctx.enter_context(tc.tile_pool(name="mpool", bufs=2))
            osbp = ctx.enter_context(tc.tile_pool(name="osbp", bufs=6))
            # PSUM is 8 banks of [128, 512] f32; every tile is bank-padded.
            # phase-B pairs live here (4 banks); y/mid pools (4 banks each)
            # are scoped per phase below and time-share the other 4 banks.
            ps_o = ctx.enter_context(tc.tile_pool(name="ps_o", bufs=1, space="PSUM"))

            # ---- input DMAs ----
            # vt first, then x as 8 x 256KB quarter-chunks on the sync
            # queue; step-1 quad q consumes exactly chunk (h, q)
            vt_sb = consts.tile([128, 1024], f16, name="vt", tag="vt")
            nc.sync.dma_start(out=vt_sb[:], in_=vt_d[:])
            vt_v = vt_sb[:].rearrange("p (i c r) -> p i c r", i=B_IN, c=2)
            xb = []
            for h in range(2):
                xt = xpool.tile([128, 32 * HT], f16, name=f"xb{h}", tag=f"xb{h}")
                for qd in range(4):
                    nc.sync.dma_start(
                        out=xt[:, qd * 8 * HT : (qd + 1) * 8 * HT],
                        in_=xt_d[h, qd],
                    )
                xb.append(xt)

            # gpsimd SWDGE queue: one coalesced factor DMA
            fac_sb = consts.tile([128, 1856], f16, name="fac", tag="fac")
            nc.gpsimd.dma_start(out=fac_sb[:], in_=fac_d[:])
            sp_sb = fac_sb[:, 0:64]
            dsb = fac_sb[:, 64:576]
            usb2 = fac_sb[:, 576:1600]    # usb2[32j+r, g*256+q] = U''[4g+j,r,q]
            kones = fac_sb[0:1, 1600:1856]

            # ---- Shat build on DVE: Shat_m = broadcast(S'_m) * D ----
            ssb = []
            for m in range(4):
                st = spool.tile([128, 4 * 128], f16, name=f"ss{m}", tag=f"ss{m}")
                nc.vector.tensor_mul(
                    st[:].rearrange("p (c k) -> p c k", k=CP),
                    sp_sb[:, m * B_OUT : (m + 1) * B_OUT]
                    .unsqueeze(2)
                    .broadcast_to([128, B_OUT, CP]),
                    dsb[:].rearrange("p (c k) -> p c k", k=CP),
                )
                ssb.append(st)

            # ---- per-half pipeline ----
            for h in range(2):
                with tc.tile_pool(name=f"psy{h}", bufs=1, space="PSUM") as ps_yh:
                    # step 1: y[(i,r), t] in 4 psum tiles (1 bank each), 4
                    # col slots; quad q consumes x chunks 8q..8q+7, and its
                    # 4 back-to-back matmuls hit 4 distinct banks AND 4
                    # distinct PE col-groups (concurrent subarray tiles).
                    yp = [
                        ps_yh.tile([128, HT], f32, name=f"y{h}{m}", tag=f"y{m}")
                        for m in range(4)
                    ]
                    for q in range(4):
                        for c in range(2):
                            for m in range(4):
                                j = (m + q) % 4
                                i = 4 * q + m
                                k = 2 * i + c
                                nc.tensor.matmul(
                                    yp[m][32 * j : 32 * j + 32, :],
                                    lhsT=vt_v[:, i, c, :],
                                    rhs=xb[h][:, k * HT : (k + 1) * HT],
                                    start=(c == 0),
                                    stop=(c == 1),
                                    tile_position=(0, 32 * j),
                                )

                    # y -> SBUF fp16, split across DVE and ACT
                    ysb = [
                        ypool.tile(
                            [128, HT], f16, name=f"ys{h}{m}", tag=f"ys{m}", bufs=2
                        )
                        for m in range(4)
                    ]
                    for m in range(4):
                        if m % 2 == 0:
                            nc.vector.tensor_copy(ysb[m][:], yp[m][:])
                        else:
                            nc.scalar.copy(ysb[m][:], yp[m][:])

                with tc.tile_pool(name=f"psm{h}", bufs=1, space="PSUM") as ps_mh:
                    # step 2: mid[g] += Shat_m[:, g-block]^T @ y_m.  Each mid
                    # bank is opened by a ones-matmul writing 1.0 everywhere:
                    # rank rows carry mid+1 (cancelled by U'' row 17 =
                    # -sum_r U via the 1.0 padding row), the rowsum row
                    # carries rowsum+1 (exactly what the bias needs).
                    mp = [
                        ps_mh.tile([128, HT], f32, name=f"mp{h}{g}", tag=f"mp{g}")
                        for g in range(4)
                    ]
                    for g in range(4):
                        nc.tensor.matmul(
                            mp[g][:],
                            lhsT=kones[:, 0:128],
                            rhs=kones[:, 0:HT],
                            start=True,
                            stop=False,
                            tile_position=(0, 0),
                        )
                    for m in range(4):
                        for g in range(4):
                            nc.tensor.matmul(
                                mp[g][:],
                                lhsT=ssb[m][:, g * 128 : (g + 1) * 128],
                                rhs=ysb[m][:],
                                start=False,
                                stop=(m == 3),
                                tile_position=(0, 0),
                            )

                    mids = [
                        mpool.tile(
                            [128, HT], f16, name=f"ms{h}{g}", tag=f"ms{g}", bufs=2
                        )
                        for g in range(4)
                    ]
                    for g in range(4):
                        if g % 2 == 0:
                            nc.vector.tensor_copy(mids[g][:], mp[g][:])
                        else:
                            nc.scalar.copy(mids[g][:], mp[g][:])

                # phase B: out tiles [128 tok, 256 q], K=18 row-tiled; the 4
                # j-slots of one mids tile run concurrently, one psum bank
                # each; per-g 256KB chunks stream out, sync queue for even
                # g, scalar for odd.
                osb_t = osbp.tile(
                    [128, B_OUT * BLOCK], f16, name=f"osb{h}", tag="osb", bufs=2
                )
                for g in range(4):
                    pos = [
                        ps_o.tile([128, BLOCK], f32, name=f"po{j}", tag=f"po{j}")
                        for j in range(4)
                    ]
                    for j in range(4):
                        nc.tensor.matmul(
                            pos[j][:],
                            lhsT=mids[g][32 * j : 32 * j + KU, :],
                            rhs=usb2[32 * j : 32 * j + KU, g * BLOCK : (g + 1) * BLOCK],
                            start=True,
                            stop=True,
                            tile_position=(32 * j, 0),
                        )
                    for j in range(4):
                        if j % 2 == 0:
                            nc.vector.tensor_copy(
                                osb_t[:, (4 * g + j) * BLOCK : (4 * g + j + 1) * BLOCK],
                                pos[j][:],
                            )
                        else:
                            nc.scalar.copy(
                                osb_t[:, (4 * g + j) * BLOCK : (4 * g + j + 1) * BLOCK],
                                pos[j][:],
                            )
                    eng = nc.sync if g % 2 == 0 else nc.scalar
                    eng.dma_start(
                        out=out_d[h][:, g * 4 * BLOCK : (g + 1) * 4 * BLOCK],
                        in_=osb_t[:, g * 4 * BLOCK : (g + 1) * 4 * BLOCK],
                    )

    nc.compile()
    return nc


def prep_inputs(x, S, U, Vt, bias):
    """Host-side layout prep. Returns per-core input maps."""
    x = np.asarray(x, dtype=np.float32)
    S = np.asarray(S, dtype=np.float32)
    U = np.asarray(U, dtype=np.float32)
    Vt = np.asarray(Vt, dtype=np.float32)
    bias = np.asarray(bias, dtype=np.float32)

    # x -> per-core [half, quarter, p, k*HT + t], in = 128k+p, tok = 256c+128h+t
    x2 = x.reshape(TOK, IN_DIM).astype(np.float16)
    # [c, h, t, k, p] -> [c, h, p, k, t] -> quarter chunks [c, h, qd, p, kq, t]
    xt5 = x2.reshape(N_CORES, 2, HT, 32, 128).transpose(0, 1, 4, 3, 2)
    xt5 = np.ascontiguousarray(xt5).reshape(N_CORES, 2, 128, 4, 8 * HT)
    xt5 = np.ascontiguousarray(xt5.transpose(0, 1, 3, 2, 4))

    # Vt_aug: [p, (i, c, r32)]; col 16 = ones (rowsum), cols 17..31 = 0
    vt_aug = np.zeros((B_IN, BLOCK, CP), np.float32)
    vt_aug[:, :, :RANK] = Vt
    vt_aug[:, :, RANK] = 1.0
    vt_host = vt_aug.reshape(B_IN, 2, 128, CP).transpose(2, 0, 1, 3).reshape(128, -1)

    # S': sp[32j+r, 16m+o] = S[o, i(m,j), r] (r<16) with i = 4q+m,
    # j = (m+q)%4 (quad q of step 1 consumes x chunks 8q..8q+7);
    # row r=16 all ones; rest 0.
    sp = np.zeros((4, CP, 4, B_OUT), np.float32)  # [j, r32, m, o]
    for m in range(4):
        for q in range(4):
            j = (m + q) % 4
            i = 4 * q + m
            sp[j, :RANK, m, :] = S[:, i, :].T
    sp[:, RANK] = 1.0
    sp_host = sp.reshape(128, 4 * B_OUT)

    # D mask: D[p, c] = 1 if p%32 == c%32 and p%32 <= 17
    pp = np.arange(128) % CP
    cc = np.arange(512) % CP
    dmask = ((pp[:, None] == cc[None, :]) & (pp[:, None] <= RANK + 1)).astype(
        np.float32
    )

    # U'' rows: 16 rank rows, bias (x rowsum+1), -sum_r U (cancels the +1
    # mid-bank-open pollution via the constant-1.0 padding row)
    bias_row = bias.reshape(B_OUT, 1, BLOCK)
    comp_row = -U.sum(axis=1, keepdims=True)
    u_aug = np.concatenate([U, bias_row, comp_row], axis=1)  # (16, 18, 256)

    # usb2[32j+r, g*256+q] = U''[o=4g+j, r, q]
    usb2 = np.zeros((4, CP, 4, BLOCK), np.float32)
    for j in range(4):
        for g in range(4):
            usb2[j, :KU, g] = u_aug[4 * g + j]
    usb2 = usb2.reshape(128, 4 * BLOCK)

    fac = np.zeros((128, 1856), np.float32)
    fac[:, 0:64] = sp_host
    fac[:, 64:576] = dmask
    fac[:, 576:1600] = usb2
    fac[0, 1600:1856] = 1.0
    fac = np.ascontiguousarray(fac).astype(np.float16)
    vt_ship = np.ascontiguousarray(vt_host).astype(np.float16)


    in_maps = []
    for c in range(N_CORES):
        in_maps.append(
            {
                "xt": np.ascontiguousarray(xt5[c]),
                "fac": fac,
                "vt": vt_ship,
            }
        )
    return in_maps


def kernel(x, S, U, Vt, bias):
    global LAST_RESULTS
    from concourse.bass_utils import run_bass_kernel_spmd

    if "nc" not in _CACHE:
        _CACHE["nc"] = build_program()
    nc = _CACHE["nc"]

    in_maps = prep_inputs(x, S, U, Vt, bias)
    res = run_bass_kernel_spmd(
        nc, in_maps, list(range(N_CORES)), trace=TRACE, tmpdir=TRACE_DIR
    )
    LAST_RESULTS = res
    out = np.concatenate(
        [res.results[c]["out"].reshape(TPC, OUT_DIM) for c in range(N_CORES)], axis=0
    ).astype(np.float32)
    return out.reshape(2, TOK // 2, OUT_DIM)
